# revision 6
# baseline (speedup 1.0000x reference)
"""Trainium2 Bass kernel for nn_Decoder: measure-LSTM -> beat-LSTM -> linear.

Strategy (data-parallel over batch, 8 cores x 32 batch each):
  Phase A: measure LSTM scan (32 steps), fused input projection, tanh ->
           latT trace in DRAM.
  Phase B1: lm = lat @ bWih[:, :H].T + bb  per measure (batch-major GEMM),
           stored to DRAM (exploits jnp.repeat structure: computed once per
           measure instead of per beat step).
  Phase C: beat LSTM scan (512 steps).  Per step:
           psum = hT.T @ bWhhT (recurrent, f32r) + xT.T @ bWih2T (inputs),
           gates = ACT(psum + lm) ; c,h elementwise ; h transposed back via
           DVE 32x32 block transposes; tanh(h) trace -> DRAM.
  Phase D: y = tanh(h) @ linW.T + linb as one bf16 GEMM.

All matmuls run with batch (32) as the PSUM partition dim and weights
streaming as the moving operand, so weight loads are tiny (32 cols).
f32r gives full-rate fp32 matmul (moving dim 512 >= 256).
"""

import sys

for _p in ("/opt/trn_rl_repo",):
    if _p not in sys.path:
        sys.path.insert(0, _p)

import numpy as np
import ml_dtypes

B, M, S = 256, 32, 16
IN, H, O = 512, 1024, 128
G = 4 * H            # 4096
T = M * S            # 512
NCORES = 8
BL = B // NCORES     # 32 batch per core
KH = H // 128        # 8 hidden chunks
NG = G // 512        # 8 gate column chunks


def _gate_perm():
    """New gate column g -> original row of W / index of bias.

    New layout: chunk n (512 cols) = [i_n | f_n | g_n | o_n], each 128 wide,
    for hidden slice n.  Original rows: i block 0:1024, f 1024:2048, etc.
    """
    idx = np.arange(G)
    n = idx >> 9
    q = (idx >> 7) & 3
    r = idx & 127
    return q * H + n * 128 + r


def _build_nc():
    import concourse.bass as bass
    import concourse.mybir as mybir
    import concourse.tile as tile
    from concourse import bacc
    from concourse.bass import ds

    f32 = mybir.dt.float32
    f32r = mybir.dt.float32r
    bf16 = mybir.dt.bfloat16
    ACTF = mybir.ActivationFunctionType
    PSUM = bass.MemorySpace.PSUM

    nc = bacc.Bacc("TRN2", target_bir_lowering=False)

    # ---- I/O (per core) ----
    latentT_d = nc.dram_tensor("latentT", [128, 4, M, BL], bf16, kind="ExternalInput")
    inputsT_d = nc.dram_tensor("inputsT", [T, 128, BL], f32r, kind="ExternalInput")
    mWihT_d = nc.dram_tensor("mWihT", [128, 4, G], bf16, kind="ExternalInput")
    mWhhT_d = nc.dram_tensor("mWhhT", [128, KH, G], f32r, kind="ExternalInput")
    mb_rep_d = nc.dram_tensor("mb_rep", [BL, G], f32, kind="ExternalInput")
    bWih1T_d = nc.dram_tensor("bWih1T", [128, KH, G], f32r, kind="ExternalInput")
    bWih2T_d = nc.dram_tensor("bWih2T", [128, G], f32r, kind="ExternalInput")
    bb_rep_d = nc.dram_tensor("bb_rep", [BL, G], f32, kind="ExternalInput")
    bWhhT_d = nc.dram_tensor("bWhhT", [128, KH, G], f32r, kind="ExternalInput")
    linWT_d = nc.dram_tensor("linWT", [128, KH, O], bf16, kind="ExternalInput")
    linb_d = nc.dram_tensor("linb", [O, 1], f32, kind="ExternalInput")

    yT_d = nc.dram_tensor("yT", [O, T * BL], f32, kind="ExternalOutput")

    # ---- scratch DRAM ----
    latT_d = nc.dram_tensor("latT_scr", [128, KH, M, BL], f32r, kind="Internal")
    # lm rows grouped so that a measure's [32, G] block is rows [m*16,(m+1)*16)
    # of a [512, 2, G] view -> dynamic ds(t0, 16) indexing from the beat loop.
    lm_d = nc.dram_tensor("lm_scr", [T, 2, G], f32, kind="Internal")
    thT_d = nc.dram_tensor("thT_scr", [128, KH, T, BL], bf16, kind="Internal")

    with tile.TileContext(nc) as tc:

        def lstm_step(pools, extra_mms, hT_rd, hT_wr, hT_wr_r, c_sb, add_ap,
                      whhT, out_dt, out_dram_slice):
            """One LSTM step for 32 batch rows.

            extra_mms: list of (lhsT_ap, rhs_ap_fn(n_slice)) input-projection
            matmuls accumulated after the recurrent ones.
            add_ap: [32, G] SBUF AP added to psum before activations.
            out_dram_slice: target of tanh(hT) [128, KH, BL] DMA.
            """
            gppool, ewpool, hpool = pools
            hsb = ewpool.tile([BL, H], f32, tag="hsb")
            for n in range(NG):
                nsl = slice(n * 512, (n + 1) * 512)
                gp = gppool.tile([BL, 512], f32, tag="gp")
                nmm = KH + len(extra_mms)
                for k in range(KH):
                    nc.tensor.matmul(gp[:], hT_rd[:, k, :], whhT[:, k, nsl],
                                     start=(k == 0), stop=False)
                for i, (lhsT, rhs_fn) in enumerate(extra_mms):
                    nc.tensor.matmul(gp[:], lhsT, rhs_fn(nsl),
                                     start=False, stop=(i == len(extra_mms) - 1))
                gact = ewpool.tile([BL, 512], f32, tag="gact")
                nc.vector.tensor_add(gact[:], gp[:], add_ap[:, nsl])
                nc.scalar.activation(gact[:, 0:256], gact[:, 0:256], ACTF.Sigmoid)
                nc.scalar.activation(gact[:, 256:384], gact[:, 256:384], ACTF.Tanh)
                nc.scalar.activation(gact[:, 384:512], gact[:, 384:512], ACTF.Sigmoid)
                csl = slice(n * 128, (n + 1) * 128)
                t1 = ewpool.tile([BL, 128], f32, tag="t1")
                t2 = ewpool.tile([BL, 128], f32, tag="t2")
                nc.vector.tensor_mul(t1[:], gact[:, 0:128], gact[:, 256:384])
                nc.vector.tensor_mul(t2[:], gact[:, 128:256], c_sb[:, csl])
                nc.vector.tensor_add(c_sb[:, csl], t1[:], t2[:])
                tct = ewpool.tile([BL, 128], f32, tag="tct")
                nc.scalar.activation(tct[:], c_sb[:, csl], ACTF.Tanh)
                nc.vector.tensor_mul(hsb[:, csl], gact[:, 384:512], tct[:])
                for q in range(4):
                    nc.vector.transpose(
                        hT_wr[q * 32:(q + 1) * 32, n, :],
                        hsb[:, n * 128 + q * 32: n * 128 + (q + 1) * 32])
                nc.scalar.activation(hT_wr_r[:, n, :], hT_wr[:, n, :], ACTF.Copy)
            tout = ewpool.tile([128, KH, BL], out_dt, tag="tout")
            nc.scalar.activation(tout[:], hT_wr[:], ACTF.Tanh)
            nc.sync.dma_start(out=out_dram_slice, in_=tout[:])

        # ================= Phase A: measure scan =================
        with (
            tc.tile_pool(name="a_w", bufs=1) as wpool,
            tc.tile_pool(name="a_state", bufs=1) as spool,
            tc.tile_pool(name="a_gp", bufs=6, space=PSUM) as gppool,
            tc.tile_pool(name="a_ew", bufs=2) as ewpool,
            tc.tile_pool(name="a_in", bufs=3) as inpool,
        ):
            whhT = wpool.tile([128, KH, G], f32r)
            nc.sync.dma_start(out=whhT[:], in_=mWhhT_d[:])
            wihT = wpool.tile([128, 4, G], bf16)
            nc.sync.dma_start(out=wihT[:], in_=mWihT_d[:])
            mb_sb = wpool.tile([BL, G], f32)
            nc.sync.dma_start(out=mb_sb[:], in_=mb_rep_d[:])

            hT_a = spool.tile([128, KH, BL], f32)
            hT_b = spool.tile([128, KH, BL], f32)
            hT_ar = spool.tile([128, KH, BL], f32r)
            hT_br = spool.tile([128, KH, BL], f32r)
            c_sb = spool.tile([BL, H], f32)
            nc.vector.memset(hT_a[:], 0.0)
            nc.vector.memset(hT_b[:], 0.0)
            nc.vector.memset(c_sb[:], 0.0)
            nc.scalar.activation(hT_ar[:], hT_a[:], ACTF.Copy)
            nc.scalar.activation(hT_br[:], hT_b[:], ACTF.Copy)

            pools = (gppool, ewpool, ewpool)
            with tc.For_i(0, M, 2, hint_engines=(mybir.EngineType.PE,)) as m0:
                for sub in range(2):
                    lat_t = inpool.tile([128, 4, BL], bf16, tag="lat")
                    nc.sync.dma_start(out=lat_t[:],
                                      in_=latentT_d[:, :, ds(m0 + sub, 1), :])
                    extra = [(lat_t[:, kc, :],
                              (lambda nsl, kc=kc: wihT[:, kc, nsl]))
                             for kc in range(4)]
                    rd_r, wr, wr_r = ((hT_ar, hT_b, hT_br) if sub == 0
                                      else (hT_br, hT_a, hT_ar))
                    lstm_step(pools, extra, rd_r, wr, wr_r, c_sb, mb_sb, whhT,
                              f32r, latT_d[:, :, ds(m0 + sub, 1), :])

        # ================= Phase B1: lm GEMM =================
        with (
            tc.tile_pool(name="b_w", bufs=1) as wpool,
            tc.tile_pool(name="b_gp", bufs=4, space=PSUM) as gppool,
            tc.tile_pool(name="b_ew", bufs=2) as ewpool,
        ):
            w1T = wpool.tile([128, KH, G], f32r)
            nc.sync.dma_start(out=w1T[:], in_=bWih1T_d[:])
            bb_sb = wpool.tile([BL, G], f32)
            nc.sync.dma_start(out=bb_sb[:], in_=bb_rep_d[:])
            lm_view = lm_d[:].rearrange("(m x) y g -> m (x y) g", m=M)
            for m in range(M):
                latm = ewpool.tile([128, KH, BL], f32r, tag="latm")
                nc.sync.dma_start(out=latm[:], in_=latT_d[:, :, m, :])
                lmtmp = ewpool.tile([BL, G], f32, tag="lmtmp")
                for n in range(NG):
                    nsl = slice(n * 512, (n + 1) * 512)
                    gp = gppool.tile([BL, 512], f32, tag="gp")
                    for k in range(KH):
                        nc.tensor.matmul(gp[:], latm[:, k, :], w1T[:, k, nsl],
                                         start=(k == 0), stop=(k == KH - 1))
                    nc.vector.tensor_add(lmtmp[:, nsl], gp[:], bb_sb[:, nsl])
                nc.sync.dma_start(out=lm_view[m], in_=lmtmp[:])

        # ================= Phase C: beat scan =================
        with (
            tc.tile_pool(name="c_w", bufs=1) as wpool,
            tc.tile_pool(name="c_state", bufs=1) as spool,
            tc.tile_pool(name="c_gp", bufs=8, space=PSUM) as gppool,
            tc.tile_pool(name="c_ew", bufs=2) as ewpool,
            tc.tile_pool(name="c_in", bufs=3) as inpool,
            tc.tile_pool(name="c_lm", bufs=1) as lmpool,
        ):
            whhT = wpool.tile([128, KH, G], f32r)
            nc.sync.dma_start(out=whhT[:], in_=bWhhT_d[:])
            w2T = wpool.tile([128, G], f32r)
            nc.sync.dma_start(out=w2T[:], in_=bWih2T_d[:])

            hT_a = spool.tile([128, KH, BL], f32)
            hT_b = spool.tile([128, KH, BL], f32)
            hT_ar = spool.tile([128, KH, BL], f32r)
            hT_br = spool.tile([128, KH, BL], f32r)
            c_sb = spool.tile([BL, H], f32)
            nc.vector.memset(hT_a[:], 0.0)
            nc.vector.memset(hT_b[:], 0.0)
            nc.vector.memset(c_sb[:], 0.0)
            nc.scalar.activation(hT_ar[:], hT_a[:], ACTF.Copy)
            nc.scalar.activation(hT_br[:], hT_b[:], ACTF.Copy)

            pools = (gppool, ewpool, ewpool)
            with tc.For_i(0, T, 16, hint_engines=(mybir.EngineType.PE,)) as t0:
                lm_sb = lmpool.tile([BL, G], f32)
                nc.sync.dma_start(out=lm_sb[:], in_=lm_d[ds(t0, 16), :, :])
                for s in range(16):
                    xT = inpool.tile([128, BL], f32r, tag="xT")
                    nc.sync.dma_start(out=xT[:], in_=inputsT_d[ds(t0 + s, 1), :, :])
                    extra = [(xT[:], (lambda nsl: w2T[:, nsl]))]
                    rd_r, wr, wr_r = ((hT_ar, hT_b, hT_br) if s % 2 == 0
                                      else (hT_br, hT_a, hT_ar))
                    lstm_step(pools, extra, rd_r, wr, wr_r, c_sb, lm_sb, whhT,
                              bf16, thT_d[:, :, ds(t0 + s, 1), :])

        # ================= Phase D: output GEMM =================
        with (
            tc.tile_pool(name="d_w", bufs=1) as wpool,
            tc.tile_pool(name="d_gp", bufs=4, space=PSUM) as gppool,
            tc.tile_pool(name="d_ew", bufs=3) as ewpool,
        ):
            lwT = wpool.tile([128, KH, O], bf16)
            nc.sync.dma_start(out=lwT[:], in_=linWT_d[:])
            lb = wpool.tile([O, 1], f32)
            nc.sync.dma_start(out=lb[:], in_=linb_d[:])
            for tcn in range(T * BL // 512):
                rhs = ewpool.tile([128, KH, 512], bf16, tag="rhs")
                nc.sync.dma_start(out=rhs[:],
                                  in_=thT_d[:, :, tcn * 16:(tcn + 1) * 16, :])
                yp = gppool.tile([O, 512], f32, tag="yp")
                for k in range(KH):
                    nc.tensor.matmul(yp[:], lwT[:, k, :], rhs[:, k, :],
                                     start=(k == 0), stop=(k == KH - 1))
                ysb = ewpool.tile([O, 512], f32, tag="ysb")
                nc.scalar.activation(ysb[:], yp[:], ACTF.Identity, bias=lb[:])
                nc.sync.dma_start(out=yT_d[:, tcn * 512:(tcn + 1) * 512],
                                  in_=ysb[:])

    nc.compile()
    return nc


def kernel(latent, inputs, mWih, mWhh, mb, bWih, bWhh, bb, linW, linb):
    from concourse.bass_utils import run_bass_kernel_spmd

    src = _gate_perm()
    bf = ml_dtypes.bfloat16

    def wT(w, kchunks):  # [4096, D] -> [128, kchunks, G] permuted-transposed
        return np.ascontiguousarray(
            w[src].T.reshape(kchunks, 128, G).transpose(1, 0, 2))

    mWihT = wT(mWih, 4).astype(bf)
    mWhhT = wT(mWhh, KH).astype(np.float32)
    bWih1T = wT(bWih[:, :H], KH).astype(np.float32)
    bWih2T = np.ascontiguousarray(bWih[src][:, H:].T).astype(np.float32)
    bWhhT = wT(bWhh, KH).astype(np.float32)
    mb_rep = np.ascontiguousarray(np.broadcast_to(mb[src], (BL, G))).astype(np.float32)
    bb_rep = np.ascontiguousarray(np.broadcast_to(bb[src], (BL, G))).astype(np.float32)
    linWT = np.ascontiguousarray(linW.T.reshape(KH, 128, O).transpose(1, 0, 2)).astype(bf)
    linb_c = np.ascontiguousarray(linb.reshape(O, 1)).astype(np.float32)

    shared = dict(mWihT=mWihT, mWhhT=mWhhT, mb_rep=mb_rep, bWih1T=bWih1T,
                  bWih2T=bWih2T, bb_rep=bb_rep, bWhhT=bWhhT, linWT=linWT,
                  linb=linb_c)

    in_maps = []
    for c in range(NCORES):
        bsl = slice(c * BL, (c + 1) * BL)
        latT = np.ascontiguousarray(
            latent[bsl].transpose(2, 1, 0).reshape(4, 128, M, BL)
            .transpose(1, 0, 2, 3)).astype(bf)
        inpT = np.ascontiguousarray(
            inputs[bsl].reshape(BL, T, O).transpose(1, 2, 0)).astype(np.float32)
        m = dict(shared)
        m["latentT"] = latT
        m["inputsT"] = inpT
        in_maps.append(m)

    nc = _build_nc()
    res = run_bass_kernel_spmd(nc, in_maps, core_ids=list(range(NCORES)))
    if res.exec_time_ns is not None:
        print(f"HW exec time: {res.exec_time_ns} ns", flush=True)
        if res.instructions_and_trace is not None:
            print(f"trace: {res.instructions_and_trace[1]}", flush=True)
    outs = []
    for r in res.results:
        yT = r["yT"]                      # [O, T*BL], col = t*BL + b
        outs.append(yT.reshape(O, T, BL).transpose(2, 1, 0))
    return np.concatenate(outs, axis=0).astype(np.float32)
